# revision 36
# baseline (speedup 1.0000x reference)
"""KT mutual attention kernel for 8 Trainium2 NeuronCores.

Sharding: pure data-parallel over the batch dim (B=8 -> one batch per core);
the 1024x1024 projection weights are replicated to every core.

Host-side marshalling (in _make_in_maps): weights and activations are
pre-cast to bf16 and pre-tiled into the transposed SBUF layout
  xT[p, 8a+i, f] = x.T[128i+p, 128a+f]
so the device does plain contiguous DMA loads (no casts, no on-device
transposes -- concurrent xbar DMA transposes on two HWDGE queues corrupt
data on TRN2, and serialized ones gate the projections). The target mask
is pre-transposed and pre-normalized: mask'[tl, s] = mask/(hd*sum_tl mask).

Per-core device kernel (Bass/Tile, bf16 matmuls with fp32 PSUM):
  - tq = kv@Wwq.T (natural layout), tk = tgt@Wwk.T (natural)
  - softmax scales via the masked-mean-as-matmul trick:
      inner[s, e] = sum_tl mask'[s, tl] * tk[tl, e]   (PE, K=64)
      w[h, s] = sum_hd tq[s, (h, hd)] * inner[s, (h, hd)]  (DVE mul+reduce)
  - per-m-block pipeline: qT/kT e-block m is projected, then heads 2m and
    2m+1 run bmm1 -> exp(w*logits) (ACT, scale fused) -> bmm2 with the
    ones-augmented v (row 64 = softmax denominator); projection matmuls of
    the next block fill PE while ACT drains exps (keeps the PE HAM-warm)
  - denominators: rowsums gathered into free-indexed slots, spread across
    partitions with a tiny SBUF->SBUF DMA, one batched reciprocal per 4
    heads, broadcast via a host-provided selector matmul (engine ops
    require 32-aligned partition bases)
  - out = outT.T @ Wo.T + bo
  - biases arrive bf16; all-zero biases (as produced by setup_inputs) are
    detected on the host and the K=1 bias matmuls are compiled out
"""

import sys

import numpy as np

if "/opt/trn_rl_repo" not in sys.path:
    sys.path.insert(0, "/opt/trn_rl_repo")

import ml_dtypes

import concourse.bass as bass
import concourse.mybir as mybir
import concourse.tile as tile
from concourse import bacc
from concourse.bass import ts, ds
from concourse.bass_utils import run_bass_kernel_spmd

F32 = mybir.dt.float32
BF16 = mybir.dt.bfloat16

B, T, S, TL, D = 8, 512, 1024, 64, 1024
H, HD, P = 16, 64, 128
KD = D // P  # 8 contraction blocks

N_CORES = 8
DEBUG = False

_CACHED = {}


def _emit(nc: bass.Bass, tc: "tile.TileContext", use_bias: bool) -> None:
    # ---- DRAM I/O (per core); *T tensors arrive pre-tiled bf16 ----
    hidT_d = nc.dram_tensor("hidT", [P, 4 * KD, P], BF16, kind="ExternalInput").ap()
    kvT_d = nc.dram_tensor("kvT", [P, 8 * KD, P], BF16, kind="ExternalInput").ap()
    tgtT_d = nc.dram_tensor("tgtT", [P, KD, TL], BF16, kind="ExternalInput").ap()
    maskT_d = nc.dram_tensor("maskT", [TL, KD, P], BF16, kind="ExternalInput").ap()
    Wts = {
        n: nc.dram_tensor(n, [P, 8 * KD, P], BF16, kind="ExternalInput").ap()
        for n in ("WqT", "WkT", "WvT", "WwqT", "WwkT", "WoT")
    }
    bias_dram = (
        {
            n: nc.dram_tensor(n, [1, D], BF16, kind="ExternalInput").ap()
            for n in ("bq", "bk", "bv", "bwq", "bwk", "bo")
        }
        if use_bias
        else {}
    )
    sel_dram = nc.dram_tensor("sel", [4, 256], BF16, kind="ExternalInput").ap()
    out_dram = nc.dram_tensor("out", [T, D], F32, kind="ExternalOutput").ap()

    dbg = {}
    if DEBUG:
        for name, shape, dt in (
            ("d_qT", [P, KD, T], BF16),
            ("d_kT", [P, KD, S], BF16),
            ("d_tq", [P, S // P, D], BF16),
            ("d_tk", [TL, D], BF16),
            ("d_wall", [P, S // P, H], F32),
            ("d_vaug", [P, S // P, H, HD + 1], BF16),
            ("d_attn0", [P, S // P, T], BF16),
            ("d_rinv0", [4, T], BF16),
            ("d_outT", [P, KD, T], BF16),
        ):
            dbg[name] = nc.dram_tensor(name, shape, dt, kind="ExternalOutput").ap()

    import contextlib

    with contextlib.ExitStack() as ctx:
        per = ctx.enter_context(tc.tile_pool(name="per", bufs=1))
        wt = ctx.enter_context(tc.tile_pool(name="wt", bufs=3))
        biasp = ctx.enter_context(tc.tile_pool(name="biasp", bufs=2))
        scrp = ctx.enter_context(tc.tile_pool(name="scrp", bufs=2))
        attnp = ctx.enter_context(tc.tile_pool(name="attnp", bufs=3))
        rbp = ctx.enter_context(tc.tile_pool(name="rbp", bufs=2))
        osb = ctx.enter_context(tc.tile_pool(name="osb", bufs=2))
        pp_mm = ctx.enter_context(tc.tile_pool(name="pp_mm", bufs=2, space="PSUM"))
        pp_attn = ctx.enter_context(tc.tile_pool(name="pp_attn", bufs=4, space="PSUM"))
        pp_o = ctx.enter_context(tc.tile_pool(name="pp_o", bufs=2, space="PSUM"))

        # ---- constants ----
        ones_bf = per.tile([1, 512], BF16, tag="ones_bf")
        nc.gpsimd.memset(ones_bf[:], 1.0)

        # ---- input loads, split across the two HWDGE queues in
        # consumption order; phase-1 deps (wwqT+kvT) go FIRST on their
        # queues so the first projection matmul can start ~8us in;
        # hidT (phase 2) loads last ----
        def load_wT(wname, q):
            w_t = wt.tile([P, 8 * KD, P], BF16, tag="wt")
            q.dma_start(w_t[:], Wts[wname][:])
            return w_t

        # first-consumed tensors load in halves so the first tq projection
        # matmuls can start after ~1MB instead of 2MB per queue
        wwqT = wt.tile([P, 8 * KD, P], BF16, tag="wt", name="wwqT")
        nc.sync.dma_start(wwqT[:, 0:32, :], Wts["WwqT"][:, 0:32, :])
        kvT = per.tile([P, 8 * KD, P], BF16, tag="kvT")
        nc.scalar.dma_start(kvT[:, 0:32, :], kvT_d[:, 0:32, :])
        nc.sync.dma_start(wwqT[:, 32:64, :], Wts["WwqT"][:, 32:64, :])
        nc.scalar.dma_start(kvT[:, 32:64, :], kvT_d[:, 32:64, :])
        tgtT = per.tile([P, KD, TL], BF16, tag="tgtT")
        nc.sync.dma_start(tgtT[:], tgtT_d[:])
        sel_bf = per.tile([4, 256], BF16, tag="sel_bf")
        nc.sync.dma_start(sel_bf[:], sel_dram[:])
        maskT = per.tile([TL, KD, P], BF16, tag="maskT")
        nc.scalar.dma_start(maskT[:], maskT_d[:])
        wwkT = load_wT("WwkT", nc.scalar)
        wvT = load_wT("WvT", nc.sync)
        wqT = load_wT("WqT", nc.scalar)
        wkT = load_wT("WkT", nc.sync)
        woT = load_wT("WoT", nc.scalar)
        hidT = per.tile([P, 4 * KD, P], BF16, tag="hidT")
        nc.sync.dma_start(hidT[:], hidT_d[:])

        # rhs access-pattern helper: [128, na, 128] strided over a-blocks
        def rhs_r(xT, k, a0, na):
            return xT[:].rearrange("p (a i) f -> p a i f", i=KD)[:, a0 : a0 + na, k, :]

        def load_bias(bname):
            if not use_bias:
                return None
            b = biasp.tile([1, D], BF16, tag="bias")
            nc.sync.dma_start(b[:], bias_dram[bname][:])
            return b

        def bias_mm_partition(ps, b, m, nsz):
            # bias along PSUM partitions (e): lhsT = bias chunk, rhs = ones
            if b is not None:
                nc.tensor.matmul(
                    ps[0:P, 0:nsz], b[0:1, ts(m, P)], ones_bf[0:1, 0:nsz],
                    start=False, stop=True,
                )

        def bias_mm_free(ps, b, n, mp=P):
            # bias along PSUM free dim (e): lhsT = ones, rhs = bias chunk
            if b is not None:
                nc.tensor.matmul(
                    ps[0:mp, :], ones_bf[0:1, 0:mp], b[0:1, ts(n, 512)],
                    start=False, stop=True,
                )

        def last(k, b):
            return (k == KD - 1) and b is None

        # ---- persistent tiles ----
        qT = per.tile([P, KD, T], BF16, tag="qT")
        kT = per.tile([P, KD, S], BF16, tag="kT")
        tq = per.tile([P, S // P, D], BF16, tag="tq")  # natural [s, e]
        tk = per.tile([TL, D], BF16, tag="tk")  # natural [tl, e]
        v_aug = per.tile([P, S // P, H, HD + 1], BF16, tag="v_aug")
        nc.gpsimd.memset(v_aug[:, :, :, HD : HD + 1], 1.0)
        outT = per.tile([P, KD, T], BF16, tag="outT")
        w_all = per.tile([P, S // P, H], F32, tag="w_all")

        # ---- phase 1a: tq = kv @ Wwq.T (natural), tk = tgt @ Wwk.T ----
        bwq = load_bias("bwq")
        for m in range(S // P):
            for n in range(2):
                ps = pp_mm.tile([P, 512], F32, tag="mm")
                for k in range(KD):
                    nc.tensor.matmul(
                        ps[:], kvT[:, KD * m + k, :], rhs_r(wwqT, k, 4 * n, 4),
                        start=(k == 0), stop=last(k, bwq),
                    )
                bias_mm_free(ps, bwq, n)
                nc.any.tensor_copy(tq[:, m, ds(512 * n, 512)], ps[:])

        bwk = load_bias("bwk")
        for n in range(2):
            ps = pp_mm.tile([P, 512], F32, tag="mm")
            for k in range(KD):
                nc.tensor.matmul(
                    ps[0:TL, :], tgtT[:, k, :], rhs_r(wwkT, k, 4 * n, 4),
                    start=(k == 0), stop=last(k, bwk),
                )
            bias_mm_free(ps, bwk, n, mp=TL)
            nc.any.tensor_copy(tk[0:TL, ds(512 * n, 512)], ps[0:TL, :])

        # ---- v natural: v[s, e] = sum_d kv.T[d, s] * Wv.T[d, e] + bv[e] ----
        bv = load_bias("bv")

        def v_proj_chunk(n, m):
            ps = pp_mm.tile([P, 512], F32, tag="mm")
            for k in range(KD):
                nc.tensor.matmul(
                    ps[:], kvT[:, KD * m + k, :], rhs_r(wvT, k, 4 * n, 4),
                    start=(k == 0), stop=last(k, bv),
                )
            bias_mm_free(ps, bv, n)
            nc.vector.tensor_copy(
                v_aug[:, m, ds(8 * n, 8), 0:HD],
                ps[:].rearrange("p (h x) -> p h x", x=HD),
            )

        # ---- phase 1b: w[h, s] = sum_e tq[s, e] * (mask' @ tk)[s, e] ----
        # (inner-product psums use the deep pp_attn pool; v n=0 chunks are
        # interleaved so PE stays fed while DVE drains the mul+reduce)
        for sc in range(S // P):
            v_proj_chunk(0, sc)
            for n in range(2):
                ip = pp_attn.tile([P, 512], F32, tag="aps")
                nc.tensor.matmul(
                    ip[:], maskT[0:TL, sc, :], tk[0:TL, ds(512 * n, 512)],
                    start=True, stop=True,
                )
                sc_t = scrp.tile([P, 8, HD], F32, tag="scr")
                nc.vector.tensor_mul(
                    sc_t[:],
                    ip[:].rearrange("p (h x) -> p h x", x=HD),
                    tq[:, sc, ds(512 * n, 512)].rearrange("p (h x) -> p h x", x=HD),
                )
                nc.vector.tensor_reduce(
                    w_all[:, sc, ds(8 * n, 8)], sc_t[:],
                    axis=mybir.AxisListType.X, op=mybir.AluOpType.add,
                )
        if DEBUG:
            nc.sync.dma_start(dbg["d_tq"][:], tq[:])
            nc.sync.dma_start(dbg["d_tk"][:], tk[0:TL, :])
            nc.sync.dma_start(dbg["d_wall"][:], w_all[:])

        # ---- phase 2: per e-block m: project qT/kT block, then attention
        # for heads 2m, 2m+1 (bmm1 -> exp -> bmm2), interleaved ----
        bq = load_bias("bq")
        bk = load_bias("bk")

        def qT_block(m):
            ps = pp_mm.tile([P, 512], F32, tag="mm")
            for k in range(KD):
                nc.tensor.matmul(
                    ps[:], wqT[:, KD * m + k, :], rhs_r(hidT, k, 0, 4),
                    start=(k == 0), stop=last(k, bq),
                )
            bias_mm_partition(ps, bq, m, 512)
            nc.vector.tensor_copy(qT[:, m, :], ps[:])

        def kT_block(m):
            for n0 in (0, 512):
                ps = pp_mm.tile([P, 512], F32, tag="mm")
                for k in range(KD):
                    nc.tensor.matmul(
                        ps[:], wkT[:, KD * m + k, :], rhs_r(kvT, k, n0 // P, 4),
                        start=(k == 0), stop=last(k, bk),
                    )
                bias_mm_partition(ps, bk, m, 512)
                nc.vector.tensor_copy(kT[:, m, ds(n0, 512)], ps[:])

        attn_tiles = {}
        rsc_tiles = {}

        def bmm1_exp_half(h, half):
            # emit 4 of the 8 sc-chunks; splitting the group lets other PE
            # work slot in while ACT drains this half's exps (in-order queue)
            eb, eo = HD * (h % 2), h // 2
            if half == 0:
                attn_tiles[h] = attnp.tile(
                    [P, S // P, T], BF16, tag="attn", name="a_sb"
                )
            a_sb = attn_tiles[h]
            for sc in range(4 * half, 4 * half + 4):
                aps = pp_attn.tile([P, T], F32, tag="aps")
                nc.tensor.matmul(
                    aps[:], kT[eb : eb + HD, eo, ts(sc, P)], qT[eb : eb + HD, eo, :],
                    start=True, stop=True,
                )
                nc.scalar.activation(
                    a_sb[:, sc, :], aps[:],
                    mybir.ActivationFunctionType.Exp,
                    scale=w_all[:, sc, h : h + 1],
                )
            if DEBUG and h == 0 and half == 1:
                nc.sync.dma_start(dbg["d_attn0"][:], a_sb[:])

        def bmm2(h):
            eb, eo = HD * (h % 2), h // 2
            a_sb = attn_tiles.pop(h)
            ops = pp_o.tile([P, T], F32, tag="ops")
            for sc in range(S // P):
                nc.tensor.matmul(
                    ops[0 : HD + 1, :], v_aug[:, sc, h, :], a_sb[:, sc, :],
                    start=(sc == 0), stop=(sc == S // P - 1),
                )
            nc.vector.tensor_copy(outT[eb : eb + HD, eo, :], ops[0:HD, :])
            # rowsum row 64 -> free-indexed slot (partition-aligned access)
            g = h // 2
            if h % 2 == 0:
                rsc_tiles[g] = scrp.tile([1, 2, T], F32, tag="rsc", name="rsc", bufs=2)
            nc.vector.tensor_copy(rsc_tiles[g][0:1, h % 2, :], ops[HD : HD + 1, :])
            if h % 2 == 1:
                normalize_a(g)

        rinv_tiles = {}
        pending_norm = []

        def normalize_a(g):
            # head pair 2g, 2g+1: spread rowsums across 2 partitions via DMA
            # and compute batched reciprocals (no PE work -- that part is
            # deferred so the in-order PE queue is not stalled behind it)
            rsc = rsc_tiles.pop(g)
            rp = scrp.tile([2, T], F32, tag="rp", bufs=2)
            nc.sync.dma_start(rp[:], rsc[:])
            rinv2 = scrp.tile([2, T], F32, tag="rinv2", bufs=2)
            nc.vector.reciprocal_approx_fast(rinv2[:], rp[:])
            rinv_bf = scrp.tile([2, T], BF16, tag="rinv_bf", bufs=2)
            nc.vector.tensor_copy(rinv_bf[:], rinv2[:])
            rinv_tiles[g] = rinv_bf
            pending_norm.append(g)

        def normalize_b():
            # broadcast 1/rowsum via selector matmul; normalize outT in place
            # (sel rows 0/1 select rinv rows 0/1 for out halves 0-63/64-127)
            while pending_norm:
                pr = pending_norm.pop(0)
                rinv_bf = rinv_tiles.pop(pr)
                rps = pp_mm.tile([P, 512], F32, tag="mm")
                nc.tensor.matmul(
                    rps[:], sel_bf[0:2, 0:P], rinv_bf[:], start=True, stop=True
                )
                rb = rbp.tile([P, T], F32, tag="rb")
                nc.vector.tensor_copy(rb[:], rps[:])
                nc.vector.tensor_mul(
                    outT[0:HD, pr, :], outT[0:HD, pr, :], rb[0:HD, :]
                )
                nc.vector.tensor_mul(
                    outT[HD:P, pr, :], outT[HD:P, pr, :], rb[HD:P, :]
                )

        for eo in range(KD):
            qT_block(eo)
            kT_block(eo)
            bmm1_exp_half(2 * eo, 0)
            if eo >= 1:
                bmm2(2 * eo - 2)
            bmm1_exp_half(2 * eo, 1)
            bmm1_exp_half(2 * eo + 1, 0)
            if eo >= 1:
                bmm2(2 * eo - 1)
            bmm1_exp_half(2 * eo + 1, 1)
            if 1 <= eo <= 4:
                v_proj_chunk(1, 2 * (eo - 1))
                v_proj_chunk(1, 2 * (eo - 1) + 1)
            normalize_b()
        bmm2(H - 2)
        normalize_b()
        bmm2(H - 1)
        normalize_b()
        if DEBUG:
            nc.sync.dma_start(dbg["d_qT"][:], qT[:])
            nc.sync.dma_start(dbg["d_kT"][:], kT[:])
            nc.sync.dma_start(dbg["d_vaug"][:], v_aug[:])
            nc.sync.dma_start(dbg["d_outT"][:], outT[:])

        # ---- final projection: out[t, e'] = sum_e outT[e, t] WoT[e, e'] + bo ----
        bo = load_bias("bo")
        for tm in range(T // P):
            for n in range(2):
                fps = pp_mm.tile([P, 512], F32, tag="mm")
                for k in range(KD):
                    nc.tensor.matmul(
                        fps[:], outT[:, k, ts(tm, P)], rhs_r(woT, k, 4 * n, 4),
                        start=(k == 0), stop=last(k, bo),
                    )
                bias_mm_free(fps, bo, n)
                ob = osb.tile([P, 512], F32, tag="osb")
                nc.any.tensor_copy(ob[:], fps[:])
                nc.sync.dma_start(out_dram[ts(tm, P), ts(n, 512)], ob[:])


def build_nc(use_bias):
    if use_bias not in _CACHED:
        nc = bacc.Bacc("TRN2", target_bir_lowering=False, debug=False)
        with tile.TileContext(nc) as tc:
            _emit(nc, tc, use_bias)
        nc.compile()
        _CACHED[use_bias] = nc
    return _CACHED[use_bias]


def _tileT(x):
    # [rows, D] fp32 -> bf16 tiled xT[p, (a i), f] = x.T[128i+p, 128a+f]
    a = x.shape[0] // P
    return np.ascontiguousarray(
        x.reshape(a, P, KD, P).transpose(3, 0, 2, 1).reshape(P, a * KD, P)
    ).astype(ml_dtypes.bfloat16)


def _make_in_maps(inputs, use_bias):
    f = lambda t: np.asarray(t, dtype=np.float32)
    hs = f(inputs["hidden_states"])
    kvs = f(inputs["key_value_states"])
    tgt = f(inputs["target_states"])
    msk = f(inputs["target_mask"])
    shared = {}
    for wn in ("Wq", "Wk", "Wv", "Wwq", "Wwk", "Wo"):
        shared[wn + "T"] = _tileT(f(inputs[wn]))
    if use_bias:
        for bn in ("bq", "bk", "bv", "bwq", "bwk", "bo"):
            shared[bn] = f(inputs[bn]).reshape(1, D).astype(ml_dtypes.bfloat16)
    sel = np.zeros((4, 256), dtype=np.float32)
    for j in range(2):
        for p2 in range(2):
            sel[2 * j + p2, 128 * j + 64 * p2 : 128 * j + 64 * p2 + 64] = 1.0
    shared["sel"] = sel.astype(ml_dtypes.bfloat16)
    in_maps = []
    for c in range(N_CORES):
        m = dict(shared)
        m["hidT"] = _tileT(hs[c])
        m["kvT"] = _tileT(kvs[c])
        # tgtT[p, k, f] = tgt.T[128k+p, f]
        m["tgtT"] = np.ascontiguousarray(
            tgt[c].reshape(TL, KD, P).transpose(2, 1, 0)
        ).astype(ml_dtypes.bfloat16)
        # maskT[tl, sc, f] = mask[128sc+f, tl] / (hd * sum_tl mask[s, :])
        mk = msk[c, 0]  # [S, TL]
        mkn = mk / (HD * mk.sum(axis=1, keepdims=True))
        m["maskT"] = np.ascontiguousarray(
            mkn.reshape(KD, P, TL).transpose(2, 0, 1)
        ).astype(ml_dtypes.bfloat16)
        in_maps.append(m)
    return in_maps


def kernel_with_results(trace=False, **inputs):
    use_bias = any(
        np.any(np.asarray(inputs[bn])) for bn in ("bq", "bk", "bv", "bwq", "bwk", "bo")
    )
    nc = build_nc(use_bias)
    res = run_bass_kernel_spmd(
        nc,
        _make_in_maps(inputs, use_bias),
        core_ids=list(range(N_CORES)),
        trace=trace,
    )
    out = np.stack([res.results[c]["out"] for c in range(N_CORES)], axis=0)
    return out.astype(np.float32), res


def kernel(**inputs):
    out, _ = kernel_with_results(trace=False, **inputs)
    return out
